# revision 17
# baseline (speedup 1.0000x reference)
"""ConvAttention TRN2 kernel: depthwise-conv QKV + full softmax attention + projection.

Self-contained: hardcodes shapes B=2, C=96, H=W=64, N=4096, heads=3, d=32.

Sharding: each of the 8 cores computes attention for its own 512 query tokens
(q conv from a halo slice; k/v conv over the full grid redundantly per core).
The reference's reshape(B, N, C) is a scrambled reshape of [B, h, d, N], so
projection input row n is the 96-wide window attn.flat[96n : 96n+96].  Each
core projects the windows starting in its token range, using a 96-token
right-halo / 64-token left-halo of neighbor attention output obtained via one
small AllGather per batch (both hidden under later compute).  Window
extraction uses one-hot selection matmuls whose matrices are per-core *input
data*, keeping the SPMD program core-uniform.

The backend runs the PE at 1.2GHz flat, so wall time ~ PE columns; the
schedule minimizes PE work and keeps the PE stream dependency-free: conv and
projection work is interleaved between score/PV matmuls, PV lags scores by
two exp-groups, and softmax epilogues are drained lazily inside the next pair.
"""

import os
import sys

import numpy as np

for _p in ("/opt/trn_rl_repo", "/root/.axon_site/_ro/trn_rl_repo"):
    if os.path.isdir(_p) and _p not in sys.path:
        sys.path.append(_p)

from collections import deque
from contextlib import ExitStack

import concourse.bass as bass
import concourse.masks as masks
import concourse.tile as tile
from concourse import bacc, mybir
from concourse.bass_utils import run_bass_kernel_spmd

F32 = mybir.dt.float32
BF16 = mybir.dt.bfloat16

B = 2
C = 96
H = W = 64
N = H * W            # 4096
NHEADS = 3
D = C // NHEADS      # 32
SCALE = float(D) ** -0.5
NCORES = 8
NQ = 512             # query rows per core
QROWS = NQ // W      # 8 spatial rows per core
WP = W + 2           # padded width 66
NKCH = N // 128      # 32 key chunks of 128
LQ = QROWS * WP      # 528
LK = 66 * WP         # 4356
KEARLY = 10 * WP     # first 10 rows per dy plane (covers conv blocks 0-1)
EXT = 64 + NQ + 96   # ah_ext tokens: left halo | own | right halo = 672


def _build_program():
    nc = bacc.Bacc("TRN2", target_bir_lowering=False, debug=False, num_devices=NCORES)

    # host-prebuilt replicated-shift inputs: partition p = dy*32 + c holds
    # channel c shifted dy rows; partition 96 = ones (bias row)
    xq_d = nc.dram_tensor("xq", [97, B, LQ], BF16, kind="ExternalInput").ap()
    xk_d = nc.dram_tensor("xk", [97, B, LK], BF16, kind="ExternalInput").ap()
    xv_d = nc.dram_tensor("xv", [97, B, LK], BF16, kind="ExternalInput").ap()
    wm_d = nc.dram_tensor("wm", [3, 3, 97, 96], BF16, kind="ExternalInput").ap()
    cb_d = nc.dram_tensor("cb", [96, 256], BF16, kind="ExternalInput").ap()
    cf_d = nc.dram_tensor("cf", [96, 17], F32, kind="ExternalInput").ap()
    y_d = nc.dram_tensor("y", [B, 96, NQ + 64], F32, kind="ExternalOutput").ap()
    stg_d = [nc.dram_tensor(f"stg{b}", [96, 160], BF16).ap() for b in range(B)]
    gth_d = [nc.dram_tensor(f"gth{b}", [NCORES, 96, 160], BF16,
                            addr_space="Shared").ap() for b in range(B)]

    with tile.TileContext(nc) as tc, ExitStack() as ctx:
        consts = ctx.enter_context(tc.tile_pool(name="consts", bufs=1))
        xrep_p = ctx.enter_context(tc.tile_pool(name="xrep", bufs=1))
        qkv_p = ctx.enter_context(tc.tile_pool(name="qkv", bufs=1))
        exp_p = ctx.enter_context(tc.tile_pool(name="exp", bufs=4))
        small_p = ctx.enter_context(tc.tile_pool(name="small", bufs=2))

        sc_ps = ctx.enter_context(tc.tile_pool(name="sc_ps", bufs=2, space="PSUM"))
        acc_ps = ctx.enter_context(tc.tile_pool(name="acc_ps", bufs=2, space="PSUM"))
        misc_ps = ctx.enter_context(tc.tile_pool(name="misc_ps", bufs=2, space="PSUM"))

        # ---- input loads (few, large DMAs; sync + gpsimd queues only) ----
        xrep_q = xrep_p.tile([97, B, LQ], BF16)
        xrep_k = xrep_p.tile([97, B, LK], BF16)
        xrep_v = xrep_p.tile([97, B, LK], BF16)
        wm_sb = consts.tile([97, 9, 96], BF16)
        cb_sb = consts.tile([96, 256], BF16)
        cf_sb = consts.tile([96, 17], F32)

        nc.sync.dma_start(xrep_k[:, :, 0:KEARLY], xk_d[:, :, 0:KEARLY])
        nc.sync.dma_start(xrep_q[:, :, :], xq_d[:, :, :])
        nc.sync.dma_start(wm_sb[:, :, :],
                          wm_d[:, :, :, :].rearrange("g dx k o -> k (g dx) o"))
        nc.sync.dma_start(cb_sb[:, :], cb_d[:, :])
        nc.sync.dma_start(cf_sb[:, :], cf_d[:, :])
        nc.gpsimd.dma_start(xrep_k[:, :, KEARLY:LK], xk_d[:, :, KEARLY:LK])
        nc.gpsimd.dma_start(xrep_v[:, :, :], xv_d[:, :, :])

        pw_sb = cb_sb[:, 0:96]
        s3_sb = cb_sb[:, 96:192].rearrange("c (a s) -> c a s", s=32)
        sp2_sb = cb_sb[:, 192:256].rearrange("c (a s) -> c a s", s=32)
        pb_sb = cf_sb[:, 0:1]
        selr_sb = cf_sb[:, 1:9]
        sell_sb = cf_sb[:, 9:17]

        ones_col = consts.tile([1, 32], BF16)
        nc.vector.memset(ones_col[:], 1.0)
        ident = consts.tile([128, 128], BF16)
        masks.make_identity(nc, ident[:])

        # ---- persistent tiles ----
        q_all = qkv_p.tile([96, B, NQ], BF16)           # (h*32+d, b, nq)
        k_all = qkv_p.tile([96, B, N], BF16)            # (h*32+d, b, nk)
        vt_all = qkv_p.tile([128, B, NHEADS, NKCH, 33], BF16)  # (nk%128, b, h, chunk, d|1)
        ah_ext = qkv_p.tile([96, B, EXT], BF16)         # left64 | own512 | right96
        g_sb = qkv_p.tile([96, B, NCORES, 160], BF16)   # gathered halo slabs
        xt_sb = qkv_p.tile([96, B, NQ + 64], BF16)      # scrambled proj input
        ysb = qkv_p.tile([96, B, NQ + 64], F32)
        nc.vector.memset(vt_all[:, :, :, :, 32:33], 1.0)

        # warm the ACT exp table before the pipeline needs it
        dummy = small_p.tile([1, 32], BF16, tag="dummy")
        nc.scalar.activation(dummy[:, :], ones_col[:, :],
                             mybir.ActivationFunctionType.Exp, scale=SCALE)

        xq_view = [xrep_q[:, b, :].rearrange("k (r w) -> k r w", w=WP) for b in range(B)]
        xk_view = [xrep_k[:, b, :].rearrange("k (r w) -> k r w", w=WP) for b in range(B)]
        xv_view = [xrep_v[:, b, :].rearrange("k (r w) -> k r w", w=WP) for b in range(B)]

        # ---- PE work units (interleaved into the attention stream) ----
        def conv_q(b):
            ps = misc_ps.tile([128, 512], F32, tag="m")
            for dx in range(3):
                nc.tensor.matmul(
                    ps[0:96, :], lhsT=wm_sb[:, dx, :],
                    rhs=xq_view[b][:, 0:QROWS, dx: dx + W],
                    start=(dx == 0), stop=(dx == 2))
            nc.vector.tensor_copy(q_all[:, b, :], ps[0:96, :])

        def conv_k(b, blk):
            ps = misc_ps.tile([128, 512], F32, tag="m")
            for dx in range(3):
                nc.tensor.matmul(
                    ps[0:96, :], lhsT=wm_sb[:, 3 + dx, :],
                    rhs=xk_view[b][:, blk * QROWS: blk * QROWS + QROWS, dx: dx + W],
                    start=(dx == 0), stop=(dx == 2))
            nc.vector.tensor_copy(k_all[:, b, blk * 512:(blk + 1) * 512], ps[0:96, :])

        def conv_v(b, blk):
            # standard orientation + PE transposes into vt_all
            ps = misc_ps.tile([128, 512], F32, tag="m")
            for dx in range(3):
                nc.tensor.matmul(
                    ps[0:96, :], lhsT=wm_sb[:, 6 + dx, :],
                    rhs=xv_view[b][:, blk * QROWS: blk * QROWS + QROWS, dx: dx + W],
                    start=(dx == 0), stop=(dx == 2))
            vtmp = small_p.tile([96, 512], BF16, tag="vtmp")
            nc.vector.tensor_copy(vtmp[:, :], ps[0:96, :])
            tps = misc_ps.tile([128, 1024], BF16, tag="m")
            for c4 in range(4):
                nc.tensor.transpose(tps[:, c4 * 256:c4 * 256 + 96],
                                    vtmp[:, c4 * 128:(c4 + 1) * 128],
                                    ident[0:96, 0:96])
                nc.vector.tensor_copy(
                    vt_all[:, b, :, 4 * blk + c4, 0:32],
                    tps[:, c4 * 256:c4 * 256 + 96].rearrange(
                        "p (h d) -> p h d", d=32))

        # filler schedule: unit -> group -> [closures]
        fill = {}

        def add_fill(unit, grp, fn):
            fill.setdefault((unit, grp), []).append(fn)

        for j in range(2, 8):
            add_fill(0, 2 * (j - 2), lambda b=0, j=j: conv_k(b, j))
        for j in range(1, 8):
            add_fill(0, j - 1, lambda b=0, j=j: conv_v(b, j))
        for j in range(8):
            add_fill(1, 2 * j, lambda b=1, j=j: conv_k(b, j))
        add_fill(2, 0, lambda: conv_q(1))
        for j in range(8):
            add_fill(2, 2 * j + 1, lambda b=1, j=j: conv_v(b, j))

        # ---- prologue: minimum conv for pair 0's first groups ----
        conv_q(0)
        conv_k(0, 0)
        conv_k(0, 1)
        conv_v(0, 0)

        # ---- attention pair-segments, one continuous PE pipeline ----
        pend = deque()     # (acc, b, h, qseg-aware pv closure args)
        lazy = deque()     # deferred epilogues

        def pv(acc, b, h, qw, g, cpg, ex):
            for ci in range(cpg):
                ch = cpg * g + ci
                nc.tensor.matmul(
                    acc[:, 0:qw], lhsT=vt_all[:, b, h, ch, :],
                    rhs=ex[:, ci * qw:(ci + 1) * qw],
                    start=(ch == 0), stop=(ch == NKCH - 1),
                    skip_group_check=True)

        def epilogue(acc, b, h, qseg, qw):
            rden = small_p.tile([1, 512], BF16, tag="den")
            with nc.allow_low_precision(reason="softmax denom reciprocal in bf16"):
                nc.vector.reciprocal(rden[:, 0:qw], acc[32:33, 0:qw])
            bcp = misc_ps.tile([128, 512], F32, tag="m")
            nc.tensor.matmul(bcp[0:32, 0:qw], lhsT=ones_col[:, :], rhs=rden[:, 0:qw],
                             start=True, stop=True)
            num = small_p.tile([32, 512], BF16, tag="num")
            nc.vector.tensor_copy(num[:, 0:qw], acc[0:32, 0:qw])
            col = 0
            for qo, qn in qseg:
                nc.vector.tensor_mul(
                    ah_ext[32 * h:32 * (h + 1), b, 64 + qo: 64 + qo + qn],
                    num[:, col:col + qn], bcp[0:32, col:col + qn])
                col += qn

        def run_pairs():
            FULL = [(0, 512)]
            SEGA = [(0, 128), (384, 128)]
            SEGB = [(128, 256)]
            plan = [
                (0, 0, 0, FULL, None),
                (1, 0, 1, FULL, None),
                (2, 0, 2, FULL, None),
                (3, 1, 0, FULL, lambda: stage_gather(0)),
                (4, 1, 1, SEGA, None),
                (5, 1, 2, SEGA, lambda: stage_gather(1)),
                (6, 1, 1, SEGB, None),
                (7, 1, 2, SEGB, None),
            ]
            for unit, b, h, qseg, post in plan:
                qw = sum(w for _, w in qseg)
                cpg = 1024 // (2 * qw) * 2
                ngrp = NKCH // cpg
                acc = acc_ps.tile([33, 512], F32, tag="acc")
                for g in range(ngrp):
                    sc = sc_ps.tile([128, 1024], F32, tag="sc")
                    for ci in range(cpg):
                        ch = cpg * g + ci
                        col = ci * qw
                        for qo, qn in qseg:
                            nc.tensor.matmul(
                                sc[:, col:col + qn],
                                lhsT=k_all[32 * h:32 * (h + 1), b,
                                           ch * 128:(ch + 1) * 128],
                                rhs=q_all[32 * h:32 * (h + 1), b, qo:qo + qn],
                                start=True, stop=True)
                            col += qn
                    ex = exp_p.tile([128, 1024], BF16)
                    nc.scalar.activation(ex[:, :], sc[:, :],
                                         mybir.ActivationFunctionType.Exp,
                                         scale=SCALE)
                    pend.append((acc, b, h, qw, g, cpg, ex,
                                 (b, h, list(qseg), qw) if g == ngrp - 1 else None))
                    for fn in fill.get((unit, g), ()):
                        fn()
                    if lazy:
                        lazy.popleft()()
                    if len(pend) > 2:
                        args = pend.popleft()
                        pv(*args[:7])
                        if args[7]:
                            eb, eh, eqseg, eqw = args[7]
                            ea = args[0]
                            lazy.append(lambda a=ea, x=eb, y=eh, z=eqseg, w=eqw:
                                        epilogue(a, x, y, z, w))
                if post:
                    flush()
                    post()
            flush()

        def flush():
            while pend:
                args = pend.popleft()
                pv(*args[:7])
                if args[7]:
                    eb, eh, eqseg, eqw = args[7]
                    epilogue(args[0], eb, eh, eqseg, eqw)
            while lazy:
                lazy.popleft()()

        # ---- halo gather machinery ----
        def stage_gather(b):
            nc.sync.dma_start(stg_d[b][:, 0:96], ah_ext[:, b, 64:160])
            nc.sync.dma_start(stg_d[b][:, 96:160], ah_ext[:, b, 512:576])
            nc.gpsimd.collective_compute(
                "AllGather", mybir.AluOpType.bypass,
                ins=[stg_d[b][:, :]],
                outs=[gth_d[b][:, :, :]],
                replica_groups=[list(range(NCORES))])
            nc.sync.dma_start(g_sb[:, b, :, :],
                              gth_d[b][:, :, :].rearrange("j c t -> c j t"))

        def chains(b):
            nc.vector.memset(ah_ext[:, b, 576:672], 0.0)
            nc.vector.memset(ah_ext[:, b, 0:64], 0.0)
            for j in range(NCORES):
                nc.vector.scalar_tensor_tensor(
                    ah_ext[:, b, 576:672], in0=g_sb[:, b, j, 0:96],
                    scalar=selr_sb[:, j:j + 1], in1=ah_ext[:, b, 576:672],
                    op0=mybir.AluOpType.mult, op1=mybir.AluOpType.add)
                nc.vector.scalar_tensor_tensor(
                    ah_ext[:, b, 0:64], in0=g_sb[:, b, j, 96:160],
                    scalar=sell_sb[:, j:j + 1], in1=ah_ext[:, b, 0:64],
                    op0=mybir.AluOpType.mult, op1=mybir.AluOpType.add)

        def project(b):
            xtp = misc_ps.tile([128, 512], F32, tag="m")
            for j in range(16):
                nc.tensor.matmul(
                    xtp[0:96, 32 * j:32 * j + 32],
                    lhsT=ah_ext[:, b, 64 + 32 * j: 64 + 32 * j + 96],
                    rhs=s3_sb[:, j % 3, :], start=True, stop=True)
            xtw = misc_ps.tile([128, 512], F32, tag="m")
            for w in range(2):
                sa = small_p.tile([96, 96], BF16, tag="sa")
                sb_ = small_p.tile([96, 96], BF16, tag="sb")
                nc.vector.memset(sa[:, :], 0.0)
                nc.vector.memset(sb_[:, :], 0.0)
                nc.vector.tensor_copy(sa[:, 0:64 - 32 * w],
                                      ah_ext[:, b, 32 * w: 64])
                nc.vector.tensor_copy(sb_[:, 64 - 32 * w:96],
                                      ah_ext[:, b, 64: 96 + 32 * w])
                nc.tensor.matmul(xtw[0:96, 32 * w:32 * w + 32], lhsT=sa[:, :],
                                 rhs=s3_sb[:, w, :], start=True, stop=False)
                nc.tensor.matmul(xtw[0:96, 32 * w:32 * w + 32], lhsT=sb_[:, :],
                                 rhs=sp2_sb[:, w, :], start=False, stop=True)
            nc.vector.tensor_copy(xt_sb[:, b, 0:512], xtp[0:96, :])
            nc.vector.tensor_copy(xt_sb[:, b, 512:576], xtw[0:96, 0:64])
            yps = misc_ps.tile([128, 512], F32, tag="m")
            nc.tensor.matmul(yps[0:96, :], lhsT=pw_sb[:, :], rhs=xt_sb[:, b, 0:512],
                             start=True, stop=True)
            nc.vector.tensor_scalar_add(ysb[:, b, 0:512], yps[0:96, :], pb_sb[:, :])
            ypw = misc_ps.tile([128, 512], F32, tag="m")
            nc.tensor.matmul(ypw[0:96, 0:64], lhsT=pw_sb[:, :],
                             rhs=xt_sb[:, b, 512:576], start=True, stop=True)
            nc.vector.tensor_scalar_add(ysb[:, b, 512:576], ypw[0:96, 0:64],
                                        pb_sb[:, :])
            nc.sync.dma_start(y_d[b], ysb[:, b, :])

        # b0's projection runs as filler late in the b1 pairs (its gather,
        # issued at unit 3, completes during units 4-5)
        add_fill(6, 2, lambda: chains(0))
        add_fill(6, 4, lambda: project(0))

        run_pairs()
        chains(1)
        project(1)

    nc.compile()
    return nc


_PROG = None


def _prep_inputs(x, qkv_w, qkv_b, proj_w, proj_b):
    import ml_dtypes
    bf16 = ml_dtypes.bfloat16

    x = np.asarray(x, np.float32)
    qkv_w = np.asarray(qkv_w, np.float32)
    qkv_b = np.asarray(qkv_b, np.float32)
    proj_w = np.asarray(proj_w, np.float32)
    proj_b = np.asarray(proj_b, np.float32)

    xt = x.transpose(0, 2, 1).reshape(B, C, H, W)
    xpad = np.zeros((B, C, H + 4, WP), np.float32)
    xpad[:, :, 1:H + 1, 1:W + 1] = xt

    def rep_shift(ch0, row0, nrows):
        # [97, B, nrows*WP]: partition dy*32+c = channel ch0+c shifted dy rows
        out = np.ones((97, B, nrows * WP), np.float32)
        for dy in range(3):
            sl = xpad[:, ch0:ch0 + 32, row0 + dy: row0 + dy + nrows, :]
            out[dy * 32:(dy + 1) * 32] = sl.transpose(1, 0, 2, 3).reshape(
                32, B, nrows * WP)
        return out.astype(bf16)

    xks = rep_shift(32, 0, 66)
    xvs = rep_shift(64, 0, 66)
    xqs = [rep_shift(0, i * QROWS, QROWS) for i in range(NCORES)]

    w = qkv_w.reshape(3 * C, 3, 3)
    wm = np.zeros((3, 3, 97, 96), np.float32)  # [g, dx, k=(dy*32+c), o]
    o = np.arange(96)
    for g in range(3):
        for dy in range(3):
            for dx in range(3):
                wm[g, dx, dy * 32 + o // 3, o] = w[g * 96 + o, dy, dx]
        wm[g, 0, 96, :] = qkv_b[g * 96:(g + 1) * 96]
    wm = wm.astype(bf16)

    cbs, cfs = [], []
    for i in range(NCORES):
        cb = np.zeros((96, 256), np.float32)
        cb[:, 0:96] = proj_w.T
        for a in range(3):
            r = (i + a) % 3
            for s in range(32):
                cb[3 * s + r, 96 + 32 * a + s] = 1.0
        for wdx in range(2):
            for s in range(32):
                cb[3 * s + wdx + 1, 192 + 32 * wdx + s] = 1.0
        cf = np.zeros((96, 17), np.float32)
        cf[:, 0] = proj_b
        cf[:, 1 + (i + 1) % 8] = 1.0
        cf[:, 9 + (i - 1) % 8] = 1.0
        cbs.append(cb.astype(bf16))
        cfs.append(cf)
    return xqs, xks, xvs, wm, cbs, cfs


def _in_maps(inputs):
    xqs, xks, xvs, wm, cbs, cfs = _prep_inputs(
        inputs["x"], inputs["qkv_w"], inputs["qkv_b"],
        inputs["proj_w"], inputs["proj_b"])
    return [
        {"xq": xqs[i], "xk": xks, "xv": xvs, "wm": wm, "cb": cbs[i], "cf": cfs[i]}
        for i in range(NCORES)
    ]


def _col_to_n():
    """Per core: list of (column in y[b,:,0:576], output row n)."""
    maps = []
    for i in range(NCORES):
        m = []
        for j in range(16):
            if i == 7 and j >= 14:
                continue
            r = (i + j) % 3
            for s in range(32):
                n = (4096 * (3 * s + r) + 512 * i + 32 * j) // 96
                m.append((32 * j + s, n))
        if i == 0:
            for wdx in range(2):
                for s in range(32):
                    m.append((512 + 32 * wdx + s, 128 * s + 43 * wdx + 42))
        maps.append(m)
    return maps


_COLMAPS = _col_to_n()


def assemble(parts):
    """parts[i]: core i's y [B, 96, 576] -> full [B, 4096, 96]."""
    out = np.empty((B, N, 96), np.float32)
    for i, part in enumerate(parts):
        cm = _COLMAPS[i]
        cols = np.array([c for c, _ in cm])
        ns = np.array([n for _, n in cm])
        out[:, ns, :] = part[:, :, cols].transpose(0, 2, 1)
    return out


def kernel(x, qkv_w, qkv_b, proj_w, proj_b, H=64, W=64):
    global _PROG
    if _PROG is None:
        _PROG = _build_program()
    nc = _PROG

    in_maps = _in_maps({"x": x, "qkv_w": qkv_w, "qkv_b": qkv_b,
                        "proj_w": proj_w, "proj_b": proj_b})
    res = run_bass_kernel_spmd(nc, in_maps, list(range(NCORES)))
    parts = [np.asarray(res.results[i]["y"]) for i in range(NCORES)]
    return assemble(parts)


# revision 31
# speedup vs baseline: 1.0648x; 1.0648x over previous
"""ConvAttention TRN2 kernel: depthwise-conv QKV + full softmax attention + projection.

Self-contained: hardcodes shapes B=2, C=96, H=W=64, N=4096, heads=3, d=32.

Sharding: each of the 8 cores computes attention for its own 512 query tokens
(q conv from a halo slice; k/v conv over the full grid redundantly per core).
The reference's reshape(B, N, C) is a scrambled reshape of [B, h, d, N], so
projection input row n is the 96-wide window attn.flat[96n : 96n+96].  Each
core projects the windows starting in its token range, using a 96-token
right-halo / 64-token left-halo of neighbor attention output obtained via one
small AllGather per batch (both hidden under later compute).  Window
extraction uses one-hot selection matmuls whose matrices are per-core *input
data*, keeping the SPMD program core-uniform.

The backend runs the PE at 1.2GHz flat, so wall time ~ PE columns; the
schedule minimizes PE work and keeps the PE stream dependency-free: conv and
projection work is interleaved between score/PV matmuls, PV lags scores by
two exp-groups, and softmax epilogues are drained lazily inside the next pair.
"""

import os
import sys

import numpy as np

for _p in ("/opt/trn_rl_repo", "/root/.axon_site/_ro/trn_rl_repo"):
    if os.path.isdir(_p) and _p not in sys.path:
        sys.path.append(_p)

from collections import deque
from contextlib import ExitStack

import concourse.bass as bass
import concourse.masks as masks
import concourse.tile as tile
from concourse import bacc, mybir
from concourse.bass_utils import run_bass_kernel_spmd

F32 = mybir.dt.float32
BF16 = mybir.dt.bfloat16
FP8 = mybir.dt.float8e4
WSCALE = 8.0         # fp8 conv weights pre-scaled by this; drains divide it out

B = 2
C = 96
H = W = 64
N = H * W            # 4096
NHEADS = 3
D = C // NHEADS      # 32
SCALE = float(D) ** -0.5
NCORES = 8
NQ = 512             # query rows per core
QROWS = NQ // W      # 8 spatial rows per core
WP = W + 2           # padded width 66
NKCH = N // 128      # 32 key chunks of 128
LQ = QROWS * WP      # 528
LK = 66 * WP         # 4356
KEARLY = 10 * WP     # first 10 rows per dy plane (covers conv blocks 0-1)
EXT = 64 + NQ + 96   # ah_ext tokens: left halo | own | right halo = 672


def _build_program():
    nc = bacc.Bacc("TRN2", target_bir_lowering=False, debug=False, num_devices=NCORES)

    # host-prebuilt replicated-shift inputs: partition p = dy*32 + c holds
    # channel c shifted dy rows; partition 96 = ones (bias row)
    xq_d = nc.dram_tensor("xq", [97, B, LQ], BF16, kind="ExternalInput").ap()
    xk_d = nc.dram_tensor("xk", [97, B, LK], BF16, kind="ExternalInput").ap()
    xv_d = nc.dram_tensor("xv", [97, B, LK], BF16, kind="ExternalInput").ap()
    wm_d = nc.dram_tensor("wm", [3, 3, 97, 96], BF16, kind="ExternalInput").ap()
    cb_d = nc.dram_tensor("cb", [96, 256], BF16, kind="ExternalInput").ap()
    cf_d = nc.dram_tensor("cf", [96, 17], F32, kind="ExternalInput").ap()
    y_d = nc.dram_tensor("y", [B, 96, NQ + 64], F32, kind="ExternalOutput").ap()
    stg_d = [nc.dram_tensor(f"stg{b}", [96, 160], BF16).ap() for b in range(B)]
    gth_d = [nc.dram_tensor(f"gth{b}", [NCORES, 96, 160], BF16,
                            addr_space="Shared").ap() for b in range(B)]

    with tile.TileContext(nc) as tc, ExitStack() as ctx:
        consts = ctx.enter_context(tc.tile_pool(name="consts", bufs=1))
        xrep_p = ctx.enter_context(tc.tile_pool(name="xrep", bufs=1))
        qkv_p = ctx.enter_context(tc.tile_pool(name="qkv", bufs=1))
        exp_p = ctx.enter_context(tc.tile_pool(name="exp", bufs=4))
        small_p = ctx.enter_context(tc.tile_pool(name="small", bufs=2))

        sc_ps = ctx.enter_context(tc.tile_pool(name="sc_ps", bufs=2, space="PSUM"))
        acc_ps = ctx.enter_context(tc.tile_pool(name="acc_ps", bufs=2, space="PSUM"))
        misc_ps = ctx.enter_context(tc.tile_pool(name="misc_ps", bufs=2, space="PSUM"))

        # ---- input loads (gpsimd + scalar queues only: SP-queue input DMAs
        # showed erratic multi-us stalls on this runtime) ----
        xrep_q = xrep_p.tile([97, B, LQ], BF16)
        xrep_k = xrep_p.tile([97, B, LK], BF16)
        xrep_v = xrep_p.tile([97, B, LK], BF16)
        wm_sb = consts.tile([97, 9, 96], BF16)
        cb_sb = consts.tile([96, 256], BF16)
        cf_sb = consts.tile([96, 17], F32)

        nc.gpsimd.dma_start(xrep_k[:, :, 0:KEARLY], xk_d[:, :, 0:KEARLY])
        nc.gpsimd.dma_start(xrep_q[:, :, :], xq_d[:, :, :])
        nc.gpsimd.dma_start(xrep_k[:, :, KEARLY:LK], xk_d[:, :, KEARLY:LK])
        nc.gpsimd.dma_start(xrep_v[:, :, :], xv_d[:, :, :])
        nc.scalar.dma_start(wm_sb[:, :, :],
                            wm_d[:, :, :, :].rearrange("g dx k o -> k (g dx) o"))
        nc.scalar.dma_start(cb_sb[:, :], cb_d[:, :])
        nc.scalar.dma_start(cf_sb[:, :], cf_d[:, :])

        pw_sb = cb_sb[:, 0:96]
        s3_sb = cb_sb[:, 96:192].rearrange("c (a s) -> c a s", s=32)
        sp2_sb = cb_sb[:, 192:256].rearrange("c (a s) -> c a s", s=32)
        pb_sb = cf_sb[:, 0:1]
        selr_sb = cf_sb[:, 1:9]
        sell_sb = cf_sb[:, 9:17]

        ones_col = consts.tile([1, 32], BF16)
        nc.vector.memset(ones_col[:], 1.0)
        ident = consts.tile([128, 128], BF16)
        masks.make_identity(nc, ident[:])

        # ---- persistent tiles ----
        q_all = qkv_p.tile([96, B, NQ], BF16)           # (h*32+d, b, nq)
        k_all = qkv_p.tile([96, B, N], BF16)            # (h*32+d, b, nk)
        vt_all = qkv_p.tile([128, B, NHEADS, NKCH, 33], BF16)  # (nk%128, b, h, chunk, d|1)
        ah_ext = qkv_p.tile([96, B, EXT], BF16)         # left64 | own512 | right96
        g_sb = qkv_p.tile([96, B, NCORES, 160], BF16)   # gathered halo slabs
        xt_sb = qkv_p.tile([96, B, NQ + 64], BF16)      # scrambled proj input
        ysb = qkv_p.tile([96, B, NQ + 64], F32)
        nc.vector.memset(vt_all[:, :, :, :, 32:33], 1.0)

        # warm the ACT exp table before the pipeline needs it
        dummy = small_p.tile([1, 32], BF16, tag="dummy")
        nc.scalar.activation(dummy[:, :], ones_col[:, :],
                             mybir.ActivationFunctionType.Exp, scale=SCALE)

        xq_view = [xrep_q[:, b, :].rearrange("k (r w) -> k r w", w=WP) for b in range(B)]
        xk_view = [xrep_k[:, b, :].rearrange("k (r w) -> k r w", w=WP) for b in range(B)]
        xv_view = [xrep_v[:, b, :].rearrange("k (r w) -> k r w", w=WP) for b in range(B)]

        # ---- PE work units (interleaved into the attention stream) ----
        def conv_q(b):
            ps = misc_ps.tile([128, 512], F32, tag="m")
            for dx in range(3):
                nc.tensor.matmul(
                    ps[0:96, :], lhsT=wm_sb[:, dx, :],
                    rhs=xq_view[b][:, 0:QROWS, dx: dx + W],
                    start=(dx == 0), stop=(dx == 2))
            nc.vector.tensor_copy(q_all[:, b, :], ps[0:96, :])

        def conv_k(b, blk):
            ps = misc_ps.tile([128, 512], F32, tag="m")
            for dx in range(3):
                nc.tensor.matmul(
                    ps[0:96, :], lhsT=wm_sb[:, 3 + dx, :],
                    rhs=xk_view[b][:, blk * QROWS: blk * QROWS + QROWS, dx: dx + W],
                    start=(dx == 0), stop=(dx == 2))
            nc.vector.tensor_copy(k_all[:, b, blk * 512:(blk + 1) * 512], ps[0:96, :])

        def conv_v(b, blk):
            # standard orientation + PE transposes into vt_all
            ps = misc_ps.tile([128, 512], F32, tag="m")
            for dx in range(3):
                nc.tensor.matmul(
                    ps[0:96, :], lhsT=wm_sb[:, 6 + dx, :],
                    rhs=xv_view[b][:, blk * QROWS: blk * QROWS + QROWS, dx: dx + W],
                    start=(dx == 0), stop=(dx == 2))
            vtmp = small_p.tile([96, 512], BF16, tag="vtmp")
            nc.vector.tensor_copy(vtmp[:, :], ps[0:96, :])
            tps = misc_ps.tile([128, 1024], BF16, tag="m")
            for c4 in range(4):
                nc.tensor.transpose(tps[:, c4 * 256:c4 * 256 + 96],
                                    vtmp[:, c4 * 128:(c4 + 1) * 128],
                                    ident[0:96, 0:96])
                nc.vector.tensor_copy(
                    vt_all[:, b, :, 4 * blk + c4, 0:32],
                    tps[:, c4 * 256:c4 * 256 + 96].rearrange(
                        "p (h d) -> p h d", d=32))

        # filler schedule: unit -> group -> [closures]
        fill = {}

        def add_fill(unit, grp, fn):
            fill.setdefault((unit, grp), []).append(fn)

        for j in range(2, 8):
            add_fill(0, 2 * (j - 2), lambda b=0, j=j: conv_k(b, j))
        for j in range(1, 8):
            add_fill(0, j - 1, lambda b=0, j=j: conv_v(b, j))
        for j in range(8):
            add_fill(1, 2 * j, lambda b=1, j=j: conv_k(b, j))
        add_fill(2, 0, lambda: conv_q(1))
        for j in range(8):
            add_fill(2, 2 * j + 1, lambda b=1, j=j: conv_v(b, j))

        # ---- prologue: minimum conv for pair 0's first groups ----
        conv_q(0)
        conv_k(0, 0)
        conv_k(0, 1)
        conv_v(0, 0)

        # ---- attention pair-segments, one continuous PE pipeline ----
        pend = deque()     # (acc, b, h, qseg-aware pv closure args)
        lazy = deque()     # deferred epilogues

        def pv(acc, b, h, qw, g, cpg, ex):
            for ci in range(cpg):
                ch = cpg * g + ci
                nc.tensor.matmul(
                    acc[:, 0:qw], lhsT=vt_all[:, b, h, ch, :],
                    rhs=ex[:, ci * qw:(ci + 1) * qw],
                    start=(ch == 0), stop=(ch == NKCH - 1),
                    skip_group_check=True)

        def epilogue(acc, b, h, qseg, qw):
            rden = small_p.tile([1, 512], BF16, tag="den")
            with nc.allow_low_precision(reason="softmax denom reciprocal in bf16"):
                nc.vector.reciprocal(rden[:, 0:qw], acc[32:33, 0:qw])
            bcp = misc_ps.tile([128, 512], F32, tag="m")
            nc.tensor.matmul(bcp[0:32, 0:qw], lhsT=ones_col[:, :], rhs=rden[:, 0:qw],
                             start=True, stop=True)
            num = small_p.tile([32, 512], BF16, tag="num")
            nc.vector.tensor_copy(num[:, 0:qw], acc[0:32, 0:qw])
            col = 0
            for qo, qn in qseg:
                nc.vector.tensor_mul(
                    ah_ext[32 * h:32 * (h + 1), b, 64 + qo: 64 + qo + qn],
                    num[:, col:col + qn], bcp[0:32, col:col + qn])
                col += qn

        def run_pairs():
            FULL = [(0, 512)]
            SEGA = [(0, 128), (384, 128)]
            SEGB = [(128, 256)]
            plan = [
                (0, 0, 0, FULL, None),
                (1, 0, 1, FULL, None),
                (2, 0, 2, FULL, None),
                (3, 1, 0, FULL, lambda: stage_gather(0)),
                (4, 1, 1, SEGA, None),
                (5, 1, 2, SEGA, lambda: stage_gather(1)),
                (6, 1, 1, SEGB, None),
                (7, 1, 2, SEGB, None),
            ]
            for unit, b, h, qseg, post in plan:
                qw = sum(w for _, w in qseg)
                cpg = 1024 // (2 * qw) * 2
                ngrp = NKCH // cpg
                acc = acc_ps.tile([33, 512], F32, tag="acc")
                for g in range(ngrp):
                    sc = sc_ps.tile([128, 1024], F32, tag="sc")
                    for ci in range(cpg):
                        ch = cpg * g + ci
                        kx = k_all[32 * h:32 * (h + 1), b, ch * 128:(ch + 1) * 128]
                        if len(qseg) == 2:  # SEGA: edge queries via strided AP
                            qv = q_all[32 * h:32 * (h + 1), b, :].rearrange(
                                "d (s c) -> d s c", c=128)[:, 0:4:3, :]
                            nc.tensor.matmul(sc[:, ci * qw:(ci + 1) * qw],
                                             lhsT=kx, rhs=qv,
                                             start=True, stop=True)
                        else:
                            col = ci * qw
                            for qo, qn in qseg:
                                nc.tensor.matmul(
                                    sc[:, col:col + qn], lhsT=kx,
                                    rhs=q_all[32 * h:32 * (h + 1), b, qo:qo + qn],
                                    start=True, stop=True)
                                col += qn
                    ex = exp_p.tile([128, 1024], BF16)
                    nc.scalar.activation(ex[:, :], sc[:, :],
                                         mybir.ActivationFunctionType.Exp,
                                         scale=SCALE)
                    pend.append((acc, b, h, qw, g, cpg, ex,
                                 (b, h, list(qseg), qw) if g == ngrp - 1 else None))
                    for fn in fill.get((unit, g), ()):
                        fn()
                    if lazy:
                        lazy.popleft()()
                    if len(pend) > 2:
                        args = pend.popleft()
                        pv(*args[:7])
                        if args[7]:
                            eb, eh, eqseg, eqw = args[7]
                            ea = args[0]
                            lazy.append(lambda a=ea, x=eb, y=eh, z=eqseg, w=eqw:
                                        epilogue(a, x, y, z, w))
                if post:
                    flush()
                    post()
            flush()

        def flush():
            while pend:
                args = pend.popleft()
                pv(*args[:7])
                if args[7]:
                    eb, eh, eqseg, eqw = args[7]
                    epilogue(args[0], eb, eh, eqseg, eqw)
            while lazy:
                lazy.popleft()()

        # ---- halo gather machinery ----
        def stage_gather(b):
            nc.sync.dma_start(stg_d[b][:, 0:96], ah_ext[:, b, 64:160])
            nc.sync.dma_start(stg_d[b][:, 96:160], ah_ext[:, b, 512:576])
            nc.gpsimd.collective_compute(
                "AllGather", mybir.AluOpType.bypass,
                ins=[stg_d[b][:, :]],
                outs=[gth_d[b][:, :, :]],
                replica_groups=[list(range(NCORES))])
            nc.sync.dma_start(g_sb[:, b, :, :],
                              gth_d[b][:, :, :].rearrange("j c t -> c j t"))

        def chains(b):
            nc.vector.memset(ah_ext[:, b, 576:672], 0.0)
            nc.vector.memset(ah_ext[:, b, 0:64], 0.0)
            for j in range(NCORES):
                nc.vector.scalar_tensor_tensor(
                    ah_ext[:, b, 576:672], in0=g_sb[:, b, j, 0:96],
                    scalar=selr_sb[:, j:j + 1], in1=ah_ext[:, b, 576:672],
                    op0=mybir.AluOpType.mult, op1=mybir.AluOpType.add)
                nc.vector.scalar_tensor_tensor(
                    ah_ext[:, b, 0:64], in0=g_sb[:, b, j, 96:160],
                    scalar=sell_sb[:, j:j + 1], in1=ah_ext[:, b, 0:64],
                    op0=mybir.AluOpType.mult, op1=mybir.AluOpType.add)

        def project(b):
            xtp = misc_ps.tile([128, 512], F32, tag="m")
            for j in range(16):
                nc.tensor.matmul(
                    xtp[0:96, 32 * j:32 * j + 32],
                    lhsT=ah_ext[:, b, 64 + 32 * j: 64 + 32 * j + 96],
                    rhs=s3_sb[:, j % 3, :], start=True, stop=True)
            xtw = misc_ps.tile([128, 512], F32, tag="m")
            for w in range(2):
                sa = small_p.tile([96, 96], BF16, tag="sa")
                sb_ = small_p.tile([96, 96], BF16, tag="sb")
                nc.vector.memset(sa[:, :], 0.0)
                nc.vector.memset(sb_[:, :], 0.0)
                nc.vector.tensor_copy(sa[:, 0:64 - 32 * w],
                                      ah_ext[:, b, 32 * w: 64])
                nc.vector.tensor_copy(sb_[:, 64 - 32 * w:96],
                                      ah_ext[:, b, 64: 96 + 32 * w])
                nc.tensor.matmul(xtw[0:96, 32 * w:32 * w + 32], lhsT=sa[:, :],
                                 rhs=s3_sb[:, w, :], start=True, stop=False)
                nc.tensor.matmul(xtw[0:96, 32 * w:32 * w + 32], lhsT=sb_[:, :],
                                 rhs=sp2_sb[:, w, :], start=False, stop=True)
            nc.vector.tensor_copy(xt_sb[:, b, 0:512], xtp[0:96, :])
            nc.vector.tensor_copy(xt_sb[:, b, 512:576], xtw[0:96, 0:64])
            yps = misc_ps.tile([128, 512], F32, tag="m")
            nc.tensor.matmul(yps[0:96, :], lhsT=pw_sb[:, :], rhs=xt_sb[:, b, 0:512],
                             start=True, stop=True)
            nc.vector.tensor_scalar_add(ysb[:, b, 0:512], yps[0:96, :], pb_sb[:, :])
            ypw = misc_ps.tile([128, 512], F32, tag="m")
            nc.tensor.matmul(ypw[0:96, 0:64], lhsT=pw_sb[:, :],
                             rhs=xt_sb[:, b, 512:576], start=True, stop=True)
            nc.vector.tensor_scalar_add(ysb[:, b, 512:576], ypw[0:96, 0:64],
                                        pb_sb[:, :])
            nc.sync.dma_start(y_d[b], ysb[:, b, :])

        # b0's projection runs as filler late in the b1 pairs (its gather,
        # issued at unit 3, completes during units 4-5)
        add_fill(6, 2, lambda: chains(0))
        add_fill(6, 4, lambda: project(0))

        run_pairs()
        chains(1)
        project(1)

    nc.compile()
    return nc


_PROG = None


def _prep_inputs(x, qkv_w, qkv_b, proj_w, proj_b):
    import ml_dtypes
    bf16 = ml_dtypes.bfloat16

    x = np.asarray(x, np.float32)
    qkv_w = np.asarray(qkv_w, np.float32)
    qkv_b = np.asarray(qkv_b, np.float32)
    proj_w = np.asarray(proj_w, np.float32)
    proj_b = np.asarray(proj_b, np.float32)

    xt = x.transpose(0, 2, 1).reshape(B, C, H, W)
    xpad = np.zeros((B, C, H + 4, WP), np.float32)
    xpad[:, :, 1:H + 1, 1:W + 1] = xt

    def rep_shift(ch0, row0, nrows):
        # [97, B, nrows*WP]: partition dy*32+c = channel ch0+c shifted dy rows
        out = np.ones((97, B, nrows * WP), np.float32)
        for dy in range(3):
            sl = xpad[:, ch0:ch0 + 32, row0 + dy: row0 + dy + nrows, :]
            out[dy * 32:(dy + 1) * 32] = sl.transpose(1, 0, 2, 3).reshape(
                32, B, nrows * WP)
        return out.astype(bf16)

    xks = rep_shift(32, 0, 66)
    xvs = rep_shift(64, 0, 66)
    xqs = [rep_shift(0, i * QROWS, QROWS) for i in range(NCORES)]

    w = qkv_w.reshape(3 * C, 3, 3)
    wm = np.zeros((3, 3, 97, 96), np.float32)  # [g, dx, k=(dy*32+c), o]
    o = np.arange(96)
    for g in range(3):
        for dy in range(3):
            for dx in range(3):
                wm[g, dx, dy * 32 + o // 3, o] = w[g * 96 + o, dy, dx]
        wm[g, 0, 96, :] = qkv_b[g * 96:(g + 1) * 96]
    wm = wm.astype(bf16)

    cbs, cfs = [], []
    for i in range(NCORES):
        cb = np.zeros((96, 256), np.float32)
        cb[:, 0:96] = proj_w.T
        for a in range(3):
            r = (i + a) % 3
            for s in range(32):
                cb[3 * s + r, 96 + 32 * a + s] = 1.0
        for wdx in range(2):
            for s in range(32):
                cb[3 * s + wdx + 1, 192 + 32 * wdx + s] = 1.0
        cf = np.zeros((96, 17), np.float32)
        cf[:, 0] = proj_b
        cf[:, 1 + (i + 1) % 8] = 1.0
        cf[:, 9 + (i - 1) % 8] = 1.0
        cbs.append(cb.astype(bf16))
        cfs.append(cf)
    return xqs, xks, xvs, wm, cbs, cfs


def _in_maps(inputs):
    xqs, xks, xvs, wm, cbs, cfs = _prep_inputs(
        inputs["x"], inputs["qkv_w"], inputs["qkv_b"],
        inputs["proj_w"], inputs["proj_b"])
    return [
        {"xq": xqs[i], "xk": xks, "xv": xvs, "wm": wm, "cb": cbs[i], "cf": cfs[i]}
        for i in range(NCORES)
    ]


def _col_to_n():
    """Per core: list of (column in y[b,:,0:576], output row n)."""
    maps = []
    for i in range(NCORES):
        m = []
        for j in range(16):
            if i == 7 and j >= 14:
                continue
            r = (i + j) % 3
            for s in range(32):
                n = (4096 * (3 * s + r) + 512 * i + 32 * j) // 96
                m.append((32 * j + s, n))
        if i == 0:
            for wdx in range(2):
                for s in range(32):
                    m.append((512 + 32 * wdx + s, 128 * s + 43 * wdx + 42))
        maps.append(m)
    return maps


_COLMAPS = _col_to_n()


def assemble(parts):
    """parts[i]: core i's y [B, 96, 576] -> full [B, 4096, 96]."""
    out = np.empty((B, N, 96), np.float32)
    for i, part in enumerate(parts):
        cm = _COLMAPS[i]
        cols = np.array([c for c, _ in cm])
        ns = np.array([n for _, n in cm])
        out[:, ns, :] = part[:, :, cols].transpose(0, 2, 1)
    return out


def kernel(x, qkv_w, qkv_b, proj_w, proj_b, H=64, W=64):
    global _PROG
    if _PROG is None:
        _PROG = _build_program()
    nc = _PROG

    in_maps = _in_maps({"x": x, "qkv_w": qkv_w, "qkv_b": qkv_b,
                        "proj_w": proj_w, "proj_b": proj_b})
    res = run_bass_kernel_spmd(nc, in_maps, list(range(NCORES)))
    parts = [np.asarray(res.results[i]["y"]) for i in range(NCORES)]
    return assemble(parts)


# revision 39
# speedup vs baseline: 1.2032x; 1.1299x over previous
"""ConvAttention TRN2 kernel: depthwise-conv QKV + full softmax attention + projection.

Self-contained: hardcodes shapes B=2, C=96, H=W=64, N=4096, heads=3, d=32.

Sharding: each of the 8 cores computes attention for its own 512 query tokens
(q conv from a halo slice; k/v conv over the full grid redundantly per core).
The reference's reshape(B, N, C) is a scrambled reshape of [B, h, d, N], so
projection input row n is the 96-wide window attn.flat[96n : 96n+96].  Each
core projects the windows starting in its token range, using a 96-token
right-halo / 64-token left-halo of neighbor attention output obtained via one
small AllGather per batch (both hidden under later compute).  Window
extraction uses one-hot selection matmuls whose matrices are per-core *input
data*, keeping the SPMD program core-uniform.

The backend runs the PE at 1.2GHz flat, so wall time ~ PE columns; the
schedule minimizes PE work and keeps the PE stream dependency-free: conv and
projection work is interleaved between score/PV matmuls, PV lags scores by
two exp-groups, and softmax epilogues are drained lazily inside the next pair.
"""

import os
import sys

import numpy as np

for _p in ("/opt/trn_rl_repo", "/root/.axon_site/_ro/trn_rl_repo"):
    if os.path.isdir(_p) and _p not in sys.path:
        sys.path.append(_p)

from collections import deque
from contextlib import ExitStack

import concourse.bass as bass
import concourse.masks as masks
import concourse.tile as tile
from concourse import bacc, mybir
from concourse.bass_utils import run_bass_kernel_spmd

F32 = mybir.dt.float32
BF16 = mybir.dt.bfloat16
FP8 = mybir.dt.float8e4
WSCALE = 8.0         # fp8 conv weights pre-scaled by this; drains divide it out

B = 2
C = 96
H = W = 64
N = H * W            # 4096
NHEADS = 3
D = C // NHEADS      # 32
SCALE = float(D) ** -0.5
NCORES = 8
NQ = 512             # query rows per core
QROWS = NQ // W      # 8 spatial rows per core
WP = W + 2           # padded width 66
NKCH = N // 128      # 32 key chunks of 128
LQ = QROWS * WP      # 528
LK = 66 * WP         # 4356
KEARLY = 10 * WP     # first 10 rows per dy plane (covers conv blocks 0-1)
EXT = 64 + NQ + 96   # ah_ext tokens: left halo | own | right halo = 672


def _build_program():
    nc = bacc.Bacc("TRN2", target_bir_lowering=False, debug=False, num_devices=NCORES)

    # compact padded inputs: channel-major [33, B, rows*66]; partition 32 is
    # the ones (bias) row.  dy-shifted SBUF planes come from 3 overlapping
    # dy-sliced loads of the same small buffer (keeps host->device bytes low).
    xk_d = nc.dram_tensor("xk", [33, B, 68 * WP], BF16, kind="ExternalInput").ap()
    xv_d = nc.dram_tensor("xv", [33, B, 68 * WP], BF16, kind="ExternalInput").ap()
    xq_d = nc.dram_tensor("xq", [33, B, 12 * WP], BF16, kind="ExternalInput").ap()
    wm_d = nc.dram_tensor("wm", [3, 3, 97, 96], BF16, kind="ExternalInput").ap()
    cb_d = nc.dram_tensor("cb", [96, 256], BF16, kind="ExternalInput").ap()
    cf_d = nc.dram_tensor("cf", [96, 17], F32, kind="ExternalInput").ap()
    y_d = nc.dram_tensor("y", [B, 96, NQ + 64], F32, kind="ExternalOutput").ap()
    stg_d = [nc.dram_tensor(f"stg{b}", [96, 160], BF16).ap() for b in range(B)]
    gth_d = [nc.dram_tensor(f"gth{b}", [NCORES, 96, 160], BF16,
                            addr_space="Shared").ap() for b in range(B)]

    with tile.TileContext(nc) as tc, ExitStack() as ctx:
        consts = ctx.enter_context(tc.tile_pool(name="consts", bufs=1))
        xrep_p = ctx.enter_context(tc.tile_pool(name="xrep", bufs=1))
        qkv_p = ctx.enter_context(tc.tile_pool(name="qkv", bufs=1))
        exp_p = ctx.enter_context(tc.tile_pool(name="exp", bufs=4))
        small_p = ctx.enter_context(tc.tile_pool(name="small", bufs=2))

        sc_ps = ctx.enter_context(tc.tile_pool(name="sc_ps", bufs=2, space="PSUM"))
        acc_ps = ctx.enter_context(tc.tile_pool(name="acc_ps", bufs=2, space="PSUM"))
        misc_ps = ctx.enter_context(tc.tile_pool(name="misc_ps", bufs=2, space="PSUM"))

        # ---- input loads (gpsimd + scalar queues only: SP-queue input DMAs
        # showed erratic multi-us stalls on this runtime) ----
        xrep_q = xrep_p.tile([97, B, LQ], BF16)
        xrep_k = xrep_p.tile([97, B, LK], BF16)
        xrep_v = xrep_p.tile([97, B, LK], BF16)
        wm_sb = consts.tile([97, 9, 96], BF16)
        cb_sb = consts.tile([96, 256], BF16)
        cf_sb = consts.tile([96, 17], F32)

        for dy in range(3):
            nc.gpsimd.dma_start(xrep_k[dy * 32:(dy + 1) * 32, :, :],
                                xk_d[0:32, :, dy * WP: dy * WP + LK])
        nc.gpsimd.dma_start(xrep_k[96:97, :, :], xk_d[32:33, :, 0:LK])
        for dy in range(3):
            nc.gpsimd.dma_start(xrep_v[dy * 32:(dy + 1) * 32, :, :],
                                xv_d[0:32, :, dy * WP: dy * WP + LK])
        nc.gpsimd.dma_start(xrep_v[96:97, :, :], xv_d[32:33, :, 0:LK])
        for dy in range(3):
            nc.scalar.dma_start(xrep_q[dy * 32:(dy + 1) * 32, :, :],
                                xq_d[0:32, :, dy * WP: dy * WP + LQ])
        nc.scalar.dma_start(xrep_q[96:97, :, :], xq_d[32:33, :, 0:LQ])
        nc.scalar.dma_start(wm_sb[:, :, :],
                            wm_d[:, :, :, :].rearrange("g dx k o -> k (g dx) o"))
        nc.scalar.dma_start(cb_sb[:, :], cb_d[:, :])
        nc.scalar.dma_start(cf_sb[:, :], cf_d[:, :])

        pw_sb = cb_sb[:, 0:96]
        s3_sb = cb_sb[:, 96:192].rearrange("c (a s) -> c a s", s=32)
        sp2_sb = cb_sb[:, 192:256].rearrange("c (a s) -> c a s", s=32)
        pb_sb = cf_sb[:, 0:1]
        selr_sb = cf_sb[:, 1:9]
        sell_sb = cf_sb[:, 9:17]

        ones_col = consts.tile([1, 32], BF16)
        nc.vector.memset(ones_col[:], 1.0)
        ident = consts.tile([128, 128], BF16)
        masks.make_identity(nc, ident[:])

        # ---- persistent tiles ----
        q_all = qkv_p.tile([96, B, NQ], BF16)           # (h*32+d, b, nq)
        k_all = qkv_p.tile([96, B, N], BF16)            # (h*32+d, b, nk)
        vt_all = qkv_p.tile([128, B, NHEADS, NKCH, 33], BF16)  # (nk%128, b, h, chunk, d|1)
        ah_ext = qkv_p.tile([96, B, EXT], BF16)         # left64 | own512 | right96
        g_sb = qkv_p.tile([96, B, NCORES, 160], BF16)   # gathered halo slabs
        xt_sb = qkv_p.tile([96, B, NQ + 64], BF16)      # scrambled proj input
        ysb = qkv_p.tile([96, B, NQ + 64], F32)
        nc.vector.memset(vt_all[:, :, :, :, 32:33], 1.0)

        # warm the ACT exp table before the pipeline needs it
        dummy = small_p.tile([1, 32], BF16, tag="dummy")
        nc.scalar.activation(dummy[:, :], ones_col[:, :],
                             mybir.ActivationFunctionType.Exp, scale=SCALE)

        xq_view = [xrep_q[:, b, :].rearrange("k (r w) -> k r w", w=WP) for b in range(B)]
        xk_view = [xrep_k[:, b, :].rearrange("k (r w) -> k r w", w=WP) for b in range(B)]
        xv_view = [xrep_v[:, b, :].rearrange("k (r w) -> k r w", w=WP) for b in range(B)]

        # ---- PE work units (interleaved into the attention stream) ----
        def conv_q(b):
            ps = misc_ps.tile([128, 512], F32, tag="m")
            for dx in range(3):
                nc.tensor.matmul(
                    ps[0:96, :], lhsT=wm_sb[:, dx, :],
                    rhs=xq_view[b][:, 0:QROWS, dx: dx + W],
                    start=(dx == 0), stop=(dx == 2))
            nc.vector.tensor_copy(q_all[:, b, :], ps[0:96, :])

        def conv_k(b, blk):
            ps = misc_ps.tile([128, 512], F32, tag="m")
            for dx in range(3):
                nc.tensor.matmul(
                    ps[0:96, :], lhsT=wm_sb[:, 3 + dx, :],
                    rhs=xk_view[b][:, blk * QROWS: blk * QROWS + QROWS, dx: dx + W],
                    start=(dx == 0), stop=(dx == 2))
            nc.vector.tensor_copy(k_all[:, b, blk * 512:(blk + 1) * 512], ps[0:96, :])

        def conv_v(b, blk):
            # standard orientation + PE transposes into vt_all
            ps = misc_ps.tile([128, 512], F32, tag="m")
            for dx in range(3):
                nc.tensor.matmul(
                    ps[0:96, :], lhsT=wm_sb[:, 6 + dx, :],
                    rhs=xv_view[b][:, blk * QROWS: blk * QROWS + QROWS, dx: dx + W],
                    start=(dx == 0), stop=(dx == 2))
            vtmp = small_p.tile([96, 512], BF16, tag="vtmp")
            nc.vector.tensor_copy(vtmp[:, :], ps[0:96, :])
            tps = misc_ps.tile([128, 1024], BF16, tag="m")
            for c4 in range(4):
                nc.tensor.transpose(tps[:, c4 * 256:c4 * 256 + 96],
                                    vtmp[:, c4 * 128:(c4 + 1) * 128],
                                    ident[0:96, 0:96])
                nc.vector.tensor_copy(
                    vt_all[:, b, :, 4 * blk + c4, 0:32],
                    tps[:, c4 * 256:c4 * 256 + 96].rearrange(
                        "p (h d) -> p h d", d=32))

        # filler schedule: unit -> group -> [closures]
        fill = {}

        def add_fill(unit, grp, fn):
            fill.setdefault((unit, grp), []).append(fn)

        for j in range(2, 8):
            add_fill(0, 2 * (j - 2), lambda b=0, j=j: conv_k(b, j))
        for j in range(1, 8):
            add_fill(0, j - 1, lambda b=0, j=j: conv_v(b, j))
        for j in range(8):
            add_fill(1, 2 * j, lambda b=1, j=j: conv_k(b, j))
        add_fill(2, 0, lambda: conv_q(1))
        for j in range(8):
            add_fill(2, 2 * j + 1, lambda b=1, j=j: conv_v(b, j))

        # ---- prologue: minimum conv for pair 0's first groups ----
        conv_q(0)
        conv_k(0, 0)
        conv_k(0, 1)
        conv_v(0, 0)

        # ---- attention pair-segments, one continuous PE pipeline ----
        pend = deque()     # (acc, b, h, qseg-aware pv closure args)
        lazy = deque()     # deferred epilogues

        def pv(acc, b, h, qw, g, cpg, ex):
            for ci in range(cpg):
                ch = cpg * g + ci
                nc.tensor.matmul(
                    acc[:, 0:qw], lhsT=vt_all[:, b, h, ch, :],
                    rhs=ex[:, ci * qw:(ci + 1) * qw],
                    start=(ch == 0), stop=(ch == NKCH - 1),
                    skip_group_check=True)

        def epilogue(acc, b, h, qseg, qw):
            rden = small_p.tile([1, 512], BF16, tag="den")
            with nc.allow_low_precision(reason="softmax denom reciprocal in bf16"):
                nc.vector.reciprocal(rden[:, 0:qw], acc[32:33, 0:qw])
            bcp = misc_ps.tile([128, 512], F32, tag="m")
            nc.tensor.matmul(bcp[0:32, 0:qw], lhsT=ones_col[:, :], rhs=rden[:, 0:qw],
                             start=True, stop=True)
            num = small_p.tile([32, 512], BF16, tag="num")
            nc.vector.tensor_copy(num[:, 0:qw], acc[0:32, 0:qw])
            col = 0
            for qo, qn in qseg:
                nc.vector.tensor_mul(
                    ah_ext[32 * h:32 * (h + 1), b, 64 + qo: 64 + qo + qn],
                    num[:, col:col + qn], bcp[0:32, col:col + qn])
                col += qn

        def run_pairs():
            FULL = [(0, 512)]
            SEGA = [(0, 128), (384, 128)]
            SEGB = [(128, 256)]
            plan = [
                (0, 0, 0, FULL, None),
                (1, 0, 1, FULL, None),
                (2, 0, 2, FULL, lambda: stage_gather(0)),
                (3, 1, 0, SEGA, None),
                (4, 1, 1, SEGA, None),
                (5, 1, 2, SEGA, lambda: stage_gather(1)),
                (6, 1, 0, SEGB, None),
                (7, 1, 1, SEGB, None),
                (8, 1, 2, SEGB, None),
            ]
            for unit, b, h, qseg, post in plan:
                qw = sum(w for _, w in qseg)
                cpg = 1024 // (2 * qw) * 2
                ngrp = NKCH // cpg
                acc = acc_ps.tile([33, 512], F32, tag="acc")
                for g in range(ngrp):
                    sc = sc_ps.tile([128, 1024], F32, tag="sc")
                    for ci in range(cpg):
                        ch = cpg * g + ci
                        kx = k_all[32 * h:32 * (h + 1), b, ch * 128:(ch + 1) * 128]
                        if len(qseg) == 2:  # SEGA: edge queries via strided AP
                            qv = q_all[32 * h:32 * (h + 1), b, :].rearrange(
                                "d (s c) -> d s c", c=128)[:, 0:4:3, :]
                            nc.tensor.matmul(sc[:, ci * qw:(ci + 1) * qw],
                                             lhsT=kx, rhs=qv,
                                             start=True, stop=True)
                        else:
                            col = ci * qw
                            for qo, qn in qseg:
                                nc.tensor.matmul(
                                    sc[:, col:col + qn], lhsT=kx,
                                    rhs=q_all[32 * h:32 * (h + 1), b, qo:qo + qn],
                                    start=True, stop=True)
                                col += qn
                    ex = exp_p.tile([128, 1024], BF16)
                    nc.scalar.activation(ex[:, :], sc[:, :],
                                         mybir.ActivationFunctionType.Exp,
                                         scale=SCALE)
                    pend.append((acc, b, h, qw, g, cpg, ex,
                                 (b, h, list(qseg), qw) if g == ngrp - 1 else None))
                    for fn in fill.get((unit, g), ()):
                        fn()
                    if lazy:
                        lazy.popleft()()
                    if len(pend) > 2:
                        args = pend.popleft()
                        pv(*args[:7])
                        if args[7]:
                            eb, eh, eqseg, eqw = args[7]
                            ea = args[0]
                            lazy.append(lambda a=ea, x=eb, y=eh, z=eqseg, w=eqw:
                                        epilogue(a, x, y, z, w))
                if post:
                    flush()
                    post()
            flush()

        def flush():
            while pend:
                args = pend.popleft()
                pv(*args[:7])
                if args[7]:
                    eb, eh, eqseg, eqw = args[7]
                    epilogue(args[0], eb, eh, eqseg, eqw)
            while lazy:
                lazy.popleft()()

        # ---- halo gather machinery ----
        def stage_gather(b):
            nc.sync.dma_start(stg_d[b][:, 0:96], ah_ext[:, b, 64:160])
            nc.sync.dma_start(stg_d[b][:, 96:160], ah_ext[:, b, 512:576])
            nc.gpsimd.collective_compute(
                "AllGather", mybir.AluOpType.bypass,
                ins=[stg_d[b][:, :]],
                outs=[gth_d[b][:, :, :]],
                replica_groups=[list(range(NCORES))])
            nc.sync.dma_start(g_sb[:, b, :, :],
                              gth_d[b][:, :, :].rearrange("j c t -> c j t"))

        def chains(b):
            nc.vector.memset(ah_ext[:, b, 576:672], 0.0)
            nc.vector.memset(ah_ext[:, b, 0:64], 0.0)
            for j in range(NCORES):
                nc.vector.scalar_tensor_tensor(
                    ah_ext[:, b, 576:672], in0=g_sb[:, b, j, 0:96],
                    scalar=selr_sb[:, j:j + 1], in1=ah_ext[:, b, 576:672],
                    op0=mybir.AluOpType.mult, op1=mybir.AluOpType.add)
                nc.vector.scalar_tensor_tensor(
                    ah_ext[:, b, 0:64], in0=g_sb[:, b, j, 96:160],
                    scalar=sell_sb[:, j:j + 1], in1=ah_ext[:, b, 0:64],
                    op0=mybir.AluOpType.mult, op1=mybir.AluOpType.add)

        def project(b):
            xtp = misc_ps.tile([128, 512], F32, tag="m")
            for j in range(16):
                nc.tensor.matmul(
                    xtp[0:96, 32 * j:32 * j + 32],
                    lhsT=ah_ext[:, b, 64 + 32 * j: 64 + 32 * j + 96],
                    rhs=s3_sb[:, j % 3, :], start=True, stop=True)
            xtw = misc_ps.tile([128, 512], F32, tag="m")
            for w in range(2):
                sa = small_p.tile([96, 96], BF16, tag="sa")
                sb_ = small_p.tile([96, 96], BF16, tag="sb")
                nc.vector.memset(sa[:, :], 0.0)
                nc.vector.memset(sb_[:, :], 0.0)
                nc.vector.tensor_copy(sa[:, 0:64 - 32 * w],
                                      ah_ext[:, b, 32 * w: 64])
                nc.vector.tensor_copy(sb_[:, 64 - 32 * w:96],
                                      ah_ext[:, b, 64: 96 + 32 * w])
                nc.tensor.matmul(xtw[0:96, 32 * w:32 * w + 32], lhsT=sa[:, :],
                                 rhs=s3_sb[:, w, :], start=True, stop=False)
                nc.tensor.matmul(xtw[0:96, 32 * w:32 * w + 32], lhsT=sb_[:, :],
                                 rhs=sp2_sb[:, w, :], start=False, stop=True)
            nc.vector.tensor_copy(xt_sb[:, b, 0:512], xtp[0:96, :])
            nc.vector.tensor_copy(xt_sb[:, b, 512:576], xtw[0:96, 0:64])
            yps = misc_ps.tile([128, 512], F32, tag="m")
            nc.tensor.matmul(yps[0:96, :], lhsT=pw_sb[:, :], rhs=xt_sb[:, b, 0:512],
                             start=True, stop=True)
            nc.vector.tensor_scalar_add(ysb[:, b, 0:512], yps[0:96, :], pb_sb[:, :])
            ypw = misc_ps.tile([128, 512], F32, tag="m")
            nc.tensor.matmul(ypw[0:96, 0:64], lhsT=pw_sb[:, :],
                             rhs=xt_sb[:, b, 512:576], start=True, stop=True)
            nc.vector.tensor_scalar_add(ysb[:, b, 512:576], ypw[0:96, 0:64],
                                        pb_sb[:, :])
            nc.sync.dma_start(y_d[b], ysb[:, b, :])

        # b0's projection runs as filler late in the b1 pairs (its gather,
        # issued at unit 2's end, completes during units 3-4)
        add_fill(5, 0, lambda: chains(0))
        add_fill(5, 4, lambda: project(0))

        run_pairs()
        chains(1)
        project(1)

    nc.compile()
    return nc


_PROG = None


def _prep_inputs(x, qkv_w, qkv_b, proj_w, proj_b):
    import ml_dtypes
    bf16 = ml_dtypes.bfloat16

    x = np.asarray(x, np.float32)
    qkv_w = np.asarray(qkv_w, np.float32)
    qkv_b = np.asarray(qkv_b, np.float32)
    proj_w = np.asarray(proj_w, np.float32)
    proj_b = np.asarray(proj_b, np.float32)

    xt = x.transpose(0, 2, 1).reshape(B, C, H, W)
    xpad = np.zeros((B, C, H + 4, WP), np.float32)
    xpad[:, :, 1:H + 1, 1:W + 1] = xt

    def compact(ch0, row0, nrows):
        # [33, B, nrows*WP]: channel-major padded slab; partition 32 = ones
        out = np.ones((33, B, nrows * WP), np.float32)
        sl = xpad[:, ch0:ch0 + 32, row0: row0 + nrows, :]
        out[0:32] = sl.transpose(1, 0, 2, 3).reshape(32, B, nrows * WP)
        return out.astype(bf16)

    xks = compact(32, 0, 68)
    xvs = compact(64, 0, 68)
    xqs = [compact(0, i * QROWS, 12) for i in range(NCORES)]

    w = qkv_w.reshape(3 * C, 3, 3)
    wm = np.zeros((3, 3, 97, 96), np.float32)  # [g, dx, k=(dy*32+c), o]
    o = np.arange(96)
    for g in range(3):
        for dy in range(3):
            for dx in range(3):
                wm[g, dx, dy * 32 + o // 3, o] = w[g * 96 + o, dy, dx]
        wm[g, 0, 96, :] = qkv_b[g * 96:(g + 1) * 96]
    wm = wm.astype(bf16)

    cbs, cfs = [], []
    for i in range(NCORES):
        cb = np.zeros((96, 256), np.float32)
        cb[:, 0:96] = proj_w.T
        for a in range(3):
            r = (i + a) % 3
            for s in range(32):
                cb[3 * s + r, 96 + 32 * a + s] = 1.0
        for wdx in range(2):
            for s in range(32):
                cb[3 * s + wdx + 1, 192 + 32 * wdx + s] = 1.0
        cf = np.zeros((96, 17), np.float32)
        cf[:, 0] = proj_b
        cf[:, 1 + (i + 1) % 8] = 1.0
        cf[:, 9 + (i - 1) % 8] = 1.0
        cbs.append(cb.astype(bf16))
        cfs.append(cf)
    return xqs, xks, xvs, wm, cbs, cfs


def _in_maps(inputs):
    xqs, xks, xvs, wm, cbs, cfs = _prep_inputs(
        inputs["x"], inputs["qkv_w"], inputs["qkv_b"],
        inputs["proj_w"], inputs["proj_b"])
    return [
        {"xk": xks, "xv": xvs, "xq": xqs[i], "wm": wm, "cb": cbs[i], "cf": cfs[i]}
        for i in range(NCORES)
    ]


def _col_to_n():
    """Per core: list of (column in y[b,:,0:576], output row n)."""
    maps = []
    for i in range(NCORES):
        m = []
        for j in range(16):
            if i == 7 and j >= 14:
                continue
            r = (i + j) % 3
            for s in range(32):
                n = (4096 * (3 * s + r) + 512 * i + 32 * j) // 96
                m.append((32 * j + s, n))
        if i == 0:
            for wdx in range(2):
                for s in range(32):
                    m.append((512 + 32 * wdx + s, 128 * s + 43 * wdx + 42))
        maps.append(m)
    return maps


_COLMAPS = _col_to_n()


def assemble(parts):
    """parts[i]: core i's y [B, 96, 576] -> full [B, 4096, 96]."""
    out = np.empty((B, N, 96), np.float32)
    for i, part in enumerate(parts):
        cm = _COLMAPS[i]
        cols = np.array([c for c, _ in cm])
        ns = np.array([n for _, n in cm])
        out[:, ns, :] = part[:, :, cols].transpose(0, 2, 1)
    return out


def kernel(x, qkv_w, qkv_b, proj_w, proj_b, H=64, W=64):
    global _PROG
    if _PROG is None:
        _PROG = _build_program()
    nc = _PROG

    in_maps = _in_maps({"x": x, "qkv_w": qkv_w, "qkv_b": qkv_b,
                        "proj_w": proj_w, "proj_b": proj_b})
    res = run_bass_kernel_spmd(nc, in_maps, list(range(NCORES)))
    parts = [np.asarray(res.results[i]["y"]) for i in range(NCORES)]
    return assemble(parts)
